# revision 1
# baseline (speedup 1.0000x reference)
"""Trainium2 Bass kernel for Mistral-style sliding-window GQA attention.

Problem (hardcoded shapes):
  hidden_states [2048, 4096] f32, Wq [4096, 4096], Wk/Wv [4096, 1024],
  Wo [4096, 4096], cu_seqlens [3] int32. 32 Q heads / 8 KV heads,
  head_dim 128, sliding window 512, rope theta 10000.

Sharding: tensor-parallel over heads across 8 cores. Core c owns Q heads
[4c, 4c+4) and KV head c (GQA groups align: qh//4 == c). Wq/Wk/Wv are
column-sharded, Wo row-sharded; each core emits a partial [2048, 4096]
output which the host sums.

Device kernel layout choices (per core):
  - hT = hidden^T [4096, 2048] bf16 is the streamed rhs for all
    projections (qT/kT/vT come out in [head_dim, T] layout with weight
    tiles as the stationary operand in natural layout).
  - RoPE: rotate_half is two partition-shifted DVE multiplies against a
    sign-folded sin table; no PE work, no extra permutation matrix.
  - scores are computed transposed (ST[k,q] = kT.T @ qT) for two heads
    at once (q tiles of the head pair interleaved in SBUF), so score
    matmul N=256 and one exp per pair. Softmax skips max-subtraction
    (scores are O(10), far from fp32 exp overflow); the denominator
    comes free as a ones-column appended to V.
  - partial-tile masks (causal diagonal / window edge / arbitrary
    cu_seqlens boundaries) are host-computed 0/1 bf16 tiles (duplicated
    per head pair), applied multiplicatively after exp on GpSimd.
  - attention output [q, dim] is normalized per-partition (reciprocal of
    the ones-column), transposed on the PE into a shared PSUM bank, and
    fed as lhsT to the row-parallel Wo matmul; partials bounce
    PSUM->SBUF (DVE/ACT alternating) and stream to DRAM.
"""

import numpy as np
import ml_dtypes

import concourse.bass as bass
import concourse.tile as tile
from concourse import bacc, mybir
from concourse import bass_utils

# ---- problem constants -------------------------------------------------
T = 2048
HID = 4096
NUM_HEADS = 32
NUM_KV_HEADS = 8
D = 128  # head dim
WINDOW = 512
ROPE_THETA = 10000.0
N_CORES = 8
HPC = NUM_HEADS // N_CORES  # 4 q heads per core
QD = HPC * D  # 512 q-proj cols per core

NT = T // 128  # 16 token tiles
NKT = HID // 128  # 32 hidden k-tiles
NSTRIP = T // 512  # 4 token strips of 512
NOUT = HID // 512  # 8 output column slices

F32 = mybir.dt.float32
BF16 = mybir.dt.bfloat16
SCALE = 1.0 / np.sqrt(D)

_cache = {}


def _host_prep(cu_seqlens):
    """Everything derived from cu_seqlens: positions, rope tables,
    per-tile job list and mask tiles (ST layout [k, q], head-pair
    duplicated to [128, 256])."""
    cu = np.asarray(cu_seqlens, dtype=np.int64)
    tok = np.arange(T)
    seg = np.searchsorted(cu[1:], tok, side="right")
    pos = tok - cu[np.minimum(seg, len(cu) - 1)]

    same = seg[:, None] == seg[None, :]
    causal = pos[None, :] <= pos[:, None]
    win = pos[None, :] >= pos[:, None] - (WINDOW - 1)
    allowed = same & causal & win  # [q, k]

    jobs = []  # jobs[i] = [(j, mask_id | None), ...]
    masks = []
    mask_index = {}
    for i in range(NT):
        row = []
        for j in range(NT):
            blk = allowed[128 * i : 128 * (i + 1), 128 * j : 128 * (j + 1)]
            if not blk.any():
                continue
            if blk.all():
                row.append((j, None))
            else:
                key = blk.tobytes()
                if key not in mask_index:
                    mask_index[key] = len(masks)
                    masks.append(blk.T.astype(np.float32))  # ST layout
                row.append((j, mask_index[key]))
        jobs.append(row)
    if not masks:
        masks.append(np.ones((128, 128), np.float32))
    m = np.stack(masks)
    masks_np = np.concatenate([m, m], axis=2).astype(ml_dtypes.bfloat16)

    inv = 1.0 / (ROPE_THETA ** (np.arange(0, D, 2, dtype=np.float64) / D))
    freqs = pos[:, None].astype(np.float64) * inv[None, :]  # [T, 64]
    emb = np.concatenate([freqs, freqs], axis=1)  # [T, 128]
    cos_t = np.cos(emb).T.astype(np.float32).copy()  # [128, T]
    sin_t = np.sin(emb).T.astype(np.float32)
    # sign-folded: rope(x)[d] = x[d]*cos[d] + x[(d+64)%128] * sin_s[d]
    sin_s = np.concatenate([-sin_t[:64], sin_t[64:]], axis=0).copy()
    ident = np.eye(128, dtype=ml_dtypes.bfloat16)

    return jobs, masks_np, cos_t, sin_s, ident


def _build(jobs, n_masks):
    """Trace the Bass/Tile program (identical on all cores)."""
    nc = bacc.Bacc("TRN2", target_bir_lowering=False, debug=False,
                   num_devices=N_CORES)

    # DRAM I/O (per-core shapes)
    ht_d = nc.dram_tensor("ht", [NSTRIP, NKT // 4, 128, 2048], BF16,
                          kind="ExternalInput").ap()
    wq_d = nc.dram_tensor("wq", [HPC, 128, HID], BF16,
                          kind="ExternalInput").ap()
    wk_d = nc.dram_tensor("wk", [128, HID], BF16, kind="ExternalInput").ap()
    wv_d = nc.dram_tensor("wv", [128, HID], BF16, kind="ExternalInput").ap()
    wo_d = nc.dram_tensor("wo", [HPC, 128, HID], BF16,
                          kind="ExternalInput").ap()
    cos_d = nc.dram_tensor("cos_t", [128, T], F32, kind="ExternalInput").ap()
    sin_d = nc.dram_tensor("sin_s", [128, T], F32, kind="ExternalInput").ap()
    ident_d = nc.dram_tensor("ident", [128, 128], BF16,
                             kind="ExternalInput").ap()
    masks_d = nc.dram_tensor("masks", [n_masks, 128, 256], BF16,
                             kind="ExternalInput").ap()
    out_d = nc.dram_tensor("out", [T, HID], F32, kind="ExternalOutput").ap()

    with tile.TileContext(nc) as tc:
        with tc.tile_pool(name="persist", bufs=1) as pp:
            # resident weights / tables
            wq_sb = [pp.tile([128, HID], BF16, name=f"wq{h}") for h in range(HPC)]
            wk_sb = pp.tile([128, HID], BF16, name="wk_sb")
            wv_sb = pp.tile([128, HID], BF16, name="wv_sb")
            wo_sb = [pp.tile([128, HID], BF16, name=f"wo{h}") for h in range(HPC)]
            cos_sb = pp.tile([128, T], F32, name="cos_sb")
            sin_sb = pp.tile([128, T], F32, name="sin_sb")
            ident_sb = pp.tile([128, 128], BF16, name="ident_sb")
            mask_sb = [pp.tile([128, 256], BF16, name=f"mask{m}")
                       for m in range(n_masks)]
            # activations produced by phase 1, consumed by phase 2
            # qt pairs: [128, 2*T]; cols [256*i + 128*m : +128] = head
            # (2*hp + m), token tile i.
            qt_sb = [pp.tile([128, 2 * T], BF16, name=f"qtp{hp}")
                     for hp in range(2)]
            kt_sb = pp.tile([128, T], BF16, name="kt_sb")
            vaug_sb = [pp.tile([128, D + 1], BF16, name=f"vaug{t}")
                       for t in range(NT)]

            qt_4d = [q.rearrange("p (i m c) -> p i m c", m=2, c=128)
                     for q in qt_sb]

            for t in range(NT):
                nc.vector.memset(vaug_sb[t][:, D : D + 1], 1.0)

            # ---------------- phase 1: projections + RoPE ----------------
            with (
                tc.tile_pool(name="ht_pool", bufs=6) as htp,
                tc.tile_pool(name="rope_tmp", bufs=4) as rtp,
                tc.tile_pool(name="proj_psum", bufs=6, space="PSUM") as ppp,
                tc.tile_pool(name="util_psum", bufs=2, space="PSUM") as upp,
            ):
                def rope(s, h, src):
                    """src: fp32 PSUM [128, 512] pre-rope projection."""
                    ssl = bass.ts(s, 512)
                    if h < HPC:
                        dst = qt_4d[h // 2][:, 4 * s : 4 * s + 4, h % 2, :]
                    else:
                        dst = kt_sb[:, ssl]
                    raw = rtp.tile([128, 512], F32, tag="raw",
                                   name=f"raw{s}_{h}")
                    nc.scalar.copy(raw[:], src[:])
                    t1 = rtp.tile([128, 512], F32, tag="t1",
                                  name=f"t1_{s}_{h}")
                    nc.gpsimd.tensor_mul(t1[:], raw[:], cos_sb[:, ssl])
                    # rotate_half: walrus requires TT operands to share a
                    # start partition, so swap halves via gpsimd copies first
                    # (partition-shifted copies are legal; signs live in sin_s)
                    sw = rtp.tile([128, 512], F32, tag="sw",
                                  name=f"sw{s}_{h}")
                    nc.vector.tensor_scalar_mul(sw[0:64, :],
                                                raw[64:128, :], 1.0)
                    nc.vector.tensor_scalar_mul(sw[64:128, :],
                                                raw[0:64, :], 1.0)
                    t2 = rtp.tile([128, 512], F32, tag="t2",
                                  name=f"t2_{s}_{h}")
                    nc.vector.tensor_mul(t2[:], sw[:], sin_sb[:, ssl])
                    if h < HPC:
                        t1v = t1.rearrange("p (i c) -> p i c", c=128)
                        t2v = t2.rearrange("p (i c) -> p i c", c=128)
                    else:
                        t1v, t2v = t1[:], t2[:]
                    nc.vector.tensor_add(dst, t1v, t2v)

                def v_pipeline(s, ps_v):
                    """ps_v: vT strip PSUM -> 4 v_aug tiles [k, dim]."""
                    vts = rtp.tile([128, 512], BF16, tag="vts", name=f"vts{s}")
                    nc.vector.tensor_copy(vts[:], ps_v[:])
                    vtp = upp.tile([128, 512], BF16, tag="util", name=f"vtp{s}")
                    for tt in range(4):
                        tsl = bass.ts(tt, 128)
                        nc.tensor.transpose(vtp[:, tsl], vts[:, tsl],
                                            ident_sb[:])
                        nc.vector.tensor_copy(vaug_sb[4 * s + tt][:, 0:D],
                                              vtp[:, tsl])

                def proj_round(s, heads, preamble=None, postamble=None):
                    """One k-loop computing projections `heads` (0..3 = q,
                    4 = k, 5 = v) for strip s into len(heads) PSUM banks."""
                    ps = [ppp.tile([128, 512], F32, tag="proj",
                                   name=f"ps{s}_{h}") for h in heads]
                    wt = {4: wk_sb, 5: wv_sb}
                    for g in range(NKT // 4):
                        if preamble is not None:
                            preamble(4 * g)
                        # one DMA carries 4 hidden k-tiles side by side
                        ht_t = htp.tile([128, 2048], BF16, tag="ht",
                                        name=f"ht{s}_{g}_{heads[0]}")
                        nc.sync.dma_start(ht_t[:], ht_d[s, g])
                        if postamble is not None:
                            postamble(4 * g)
                        for j in range(4):
                            k = 4 * g + j
                            ksl = bass.ts(k, 128)
                            jsl = bass.ts(j, 512)
                            first, last = k == 0, k == NKT - 1
                            for ps_t, h in zip(ps, heads):
                                w = wq_sb[h] if h < HPC else wt[h]
                                nc.tensor.matmul(ps_t[:], w[:, ksl],
                                                 ht_t[:, jsl],
                                                 start=first, stop=last)
                    return ps

                def strip0_preamble(k):
                    # the very first matmul only needs wq0's chunk; the rest
                    # of each weight-chunk group queues behind the ht tile
                    if k % 4 == 0:
                        csl = bass.ds(128 * k, 512)
                        nc.sync.dma_start(wq_sb[0][:, csl], wq_d[0][:, csl])

                def strip0_postamble(k):
                    if k % 4 == 0:
                        csl = bass.ds(128 * k, 512)
                        for h in range(1, HPC):
                            nc.sync.dma_start(wq_sb[h][:, csl],
                                              wq_d[h][:, csl])
                        nc.sync.dma_start(wk_sb[:, csl], wk_d[:, csl])
                        nc.sync.dma_start(wv_sb[:, csl], wv_d[:, csl])


                def table_chunk(s):
                    # rope-table chunk for strip s, just before its RoPE
                    csl = bass.ts(s, 512)
                    nc.sync.dma_start(cos_sb[:, csl], cos_d[:, csl])
                    nc.sync.dma_start(sin_sb[:, csl], sin_d[:, csl])
                    if s == 0:
                        nc.sync.dma_start(ident_sb[:], ident_d[:])
                    if s == 1:
                        for m in range(n_masks):
                            nc.sync.dma_start(mask_sb[m][:], masks_d[m])

                for s in range(NSTRIP - 1):
                    ps = proj_round(s, [0, 1, 2, 3, 4, 5],
                                    preamble=strip0_preamble if s == 0 else None,
                                    postamble=strip0_postamble if s == 0 else None)
                    table_chunk(s)
                    if s >= 1:
                        # wo is only needed in phase 2; trickle it in
                        nc.sync.dma_start(wo_sb[s - 1][:], wo_d[s - 1])
                    v_pipeline(s, ps[5])
                    for h in range(HPC + 1):
                        rope(s, h, ps[h])

                # Last strip in two 3-bank rounds (hT re-streamed): round A's
                # banks drain during round B's matmuls, so phase 2's PSUM
                # pools don't stall on the phase-1 epilogue.
                s = NSTRIP - 1
                ps_a = proj_round(s, [0, 1, 4])
                table_chunk(s)
                nc.sync.dma_start(wo_sb[s - 1][:], wo_d[s - 1])
                for h in (0, 1, 4):
                    rope(s, h, ps_a[(0, 1, 4).index(h)])
                ps_b = proj_round(s, [5, 2, 3])
                nc.sync.dma_start(wo_sb[s][:], wo_d[s])
                v_pipeline(s, ps_b[0])
                for h in (2, 3):
                    rope(s, h, ps_b[(5, 2, 3).index(h)])

            # ---------------- phase 2: attention + out proj --------------
            with (
                tc.tile_pool(name="attn_sbuf", bufs=8) as asp,
                tc.tile_pool(name="attn_small", bufs=4) as asmall,
                tc.tile_pool(name="score_psum", bufs=3, space="PSUM") as spp,
                tc.tile_pool(name="oaug_psum", bufs=2, space="PSUM") as opp,
                tc.tile_pool(name="oproj_psum", bufs=3, space="PSUM") as prp,
            ):

                def oproj(i, at_list):
                    isl = bass.ts(i, 128)
                    for ns in range(NOUT):
                        po = prp.tile([128, 512], F32, tag="oproj",
                                      name=f"po{i}_{ns}")
                        for h in range(HPC):
                            nc.tensor.matmul(po[:], at_list[h][:],
                                             wo_sb[h][:, bass.ts(ns, 512)],
                                             start=(h == 0), stop=(h == HPC - 1))
                        po_sb = asp.tile([128, 512], F32, tag="posb", bufs=4,
                                         name=f"posb{i}_{ns}")
                        if ns % 2 == 0:
                            nc.vector.tensor_copy(po_sb[:], po[:])
                        else:
                            nc.scalar.copy(po_sb[:], po[:])
                        nc.sync.dma_start(out_d[isl, bass.ts(ns, 512)],
                                            po_sb[:])

                prev_at = None
                for i in range(NT):
                    at_sb = []
                    njobs = len(jobs[i])
                    for hp in range(2):
                        ps_o = [opp.tile([128, D + 1], F32, tag="oaug",
                                         name=f"pso{i}_{2 * hp + m}")
                                for m in range(2)]
                        # j-tiles in pairs: two score matmuls fill one
                        # [128,512] PSUM bank, one exp covers both, then the
                        # four PV matmuls consume quarter slices
                        jl = jobs[i]
                        for p0 in range(0, njobs, 2):
                            pair = jl[p0 : p0 + 2]
                            w = 256 * len(pair)
                            ps_s = spp.tile([128, 512], F32, tag="score",
                                            name=f"pss{i}_{hp}_{p0}")
                            for q, (j, mid) in enumerate(pair):
                                nc.tensor.matmul(
                                    ps_s[:, bass.ts(q, 256)],
                                    kt_sb[:, bass.ts(j, 128)],
                                    qt_sb[hp][:, bass.ts(i, 256)],
                                    start=True, stop=True)
                            se = asp.tile([128, 512], BF16, tag="sexp",
                                          name=f"se{i}_{hp}_{p0}")
                            nc.scalar.activation(
                                se[:, 0:w], ps_s[:, 0:w],
                                mybir.ActivationFunctionType.Exp,
                                bias=0.0, scale=float(SCALE))
                            for q, (j, mid) in enumerate(pair):
                                if mid is not None:
                                    nc.gpsimd.tensor_mul(
                                        se[:, bass.ts(q, 256)],
                                        se[:, bass.ts(q, 256)],
                                        mask_sb[mid][:])
                            for q, (j, mid) in enumerate(pair):
                                jj = p0 + q
                                for m in range(2):
                                    nc.tensor.matmul(
                                        ps_o[m][:],
                                        se[:, bass.ds(256 * q + 128 * m, 128)],
                                        vaug_sb[j][:],
                                        start=(jj == 0),
                                        stop=(jj == njobs - 1))
                        for m in range(2):
                            h = 2 * hp + m
                            recip = asmall.tile([128, 1], F32, tag="recip",
                                                name=f"rc{i}_{h}")
                            nc.vector.reciprocal(recip[:],
                                                 ps_o[m][:, D : D + 1])
                            a_n = asp.tile([128, 128], BF16, tag="anorm",
                                           name=f"an{i}_{h}")
                            nc.vector.tensor_scalar_mul(a_n[:],
                                                        ps_o[m][:, 0:D],
                                                        recip[:])
                            at_p = spp.tile([128, 128], BF16, tag="score",
                                            name=f"atp{i}_{h}")
                            nc.tensor.transpose(at_p[:], a_n[:], ident_sb[:])
                            at = asp.tile([128, 128], BF16, tag="at",
                                          bufs=10, name=f"at{i}_{h}")
                            nc.vector.tensor_copy(at[:], at_p[:])
                            at_sb.append(at)

                    if prev_at is not None:
                        oproj(i - 1, prev_at)
                    prev_at = at_sb
                oproj(NT - 1, prev_at)

    nc.compile()
    return nc


def _get_nc(cu_seqlens):
    key = np.asarray(cu_seqlens).tobytes()
    if key not in _cache:
        jobs, masks_np, cos_t, sin_s, ident = _host_prep(cu_seqlens)
        nc = _build(jobs, masks_np.shape[0])
        _cache[key] = (nc, masks_np, cos_t, sin_s, ident)
    return _cache[key]


def kernel(hidden_states, Wq, Wk, Wv, Wo, cu_seqlens):
    hidden_states = np.asarray(hidden_states)
    Wq, Wk, Wv, Wo = (np.asarray(a) for a in (Wq, Wk, Wv, Wo))
    cu_seqlens = np.asarray(cu_seqlens)
    nc, masks_np, cos_t, sin_s, ident = _get_nc(cu_seqlens)

    ht = np.ascontiguousarray(hidden_states.T).astype(ml_dtypes.bfloat16)
    # tile for contiguous DMA: [NSTRIP, NKT//4, 128, 2048] — each DMA
    # carries 4 hidden k-tiles side by side in the free dim
    ht_tiled = np.ascontiguousarray(
        ht.reshape(NKT // 4, 4, 128, NSTRIP, 512).transpose(3, 0, 2, 1, 4)
    ).reshape(NSTRIP, NKT // 4, 128, 2048)

    in_maps = []
    for c in range(N_CORES):
        wq_c = Wq[:, QD * c : QD * (c + 1)].astype(ml_dtypes.bfloat16)
        # [HPC, 128, HID]: lhsT tiles, free dim = 32 hidden k-tiles side by side
        wq_t = np.ascontiguousarray(
            wq_c.reshape(NKT, 128, HPC, 128).transpose(2, 1, 0, 3)
        ).reshape(HPC, 128, HID)
        wk_c = Wk[:, D * c : D * (c + 1)].astype(ml_dtypes.bfloat16)
        wk_t = np.ascontiguousarray(
            wk_c.reshape(NKT, 128, 128).transpose(1, 0, 2)).reshape(128, HID)
        wv_c = Wv[:, D * c : D * (c + 1)].astype(ml_dtypes.bfloat16)
        wv_t = np.ascontiguousarray(
            wv_c.reshape(NKT, 128, 128).transpose(1, 0, 2)).reshape(128, HID)
        wo_c = np.ascontiguousarray(
            Wo[QD * c : QD * (c + 1), :].astype(ml_dtypes.bfloat16)
        ).reshape(HPC, 128, HID)
        in_maps.append({
            "ht": ht_tiled, "wq": wq_t, "wk": wk_t, "wv": wv_t, "wo": wo_c,
            "cos_t": cos_t, "sin_s": sin_s, "ident": ident,
            "masks": masks_np,
        })

    res = bass_utils.run_bass_kernel_spmd(nc, in_maps,
                                          core_ids=list(range(N_CORES)))
    out = res.results[0]["out"].astype(np.float64)
    for c in range(1, N_CORES):
        out += res.results[c]["out"]
    return out.astype(np.float32)



# revision 9
# speedup vs baseline: 1.0616x; 1.0616x over previous
"""Trainium2 Bass kernel for Mistral-style sliding-window GQA attention.

Problem (hardcoded shapes):
  hidden_states [2048, 4096] f32, Wq [4096, 4096], Wk/Wv [4096, 1024],
  Wo [4096, 4096], cu_seqlens [3] int32. 32 Q heads / 8 KV heads,
  head_dim 128, sliding window 512, rope theta 10000.

Sharding: tensor-parallel over heads across 8 cores. Core c owns Q heads
[4c, 4c+4) and KV head c (GQA groups align: qh//4 == c). Wq/Wk/Wv are
column-sharded, Wo row-sharded; each core emits a partial [2048, 4096]
output which the host sums.

Device kernel layout choices (per core):
  - The four big GEMMs (q/k/v projections, out-proj) run in fp8 e4m3
    DoubleRow mode (2 k-tiles per PE instruction at 0.5 cycles/row) with
    a 3-term residual decomposition X@W ~= X8@W8 + RX8@W8 + X8@RW8.
    Operands are quantized at power-of-2 scales (X*32, W*1024, A*16)
    with residuals on the same grid, so all three terms accumulate in
    one PSUM bank and the single descale folds into existing table /
    copy steps (cos/sin tables, v copy, out-proj PSUM drain). Each bank
    fill has exactly ONE start=True (first instr) and ONE stop=True
    (last instr): start marks the whole 2KB bank pending-zero, so each
    column-half's first write lands on zeroed bytes.
  - hT = hidden^T in e4m3 (x8 ++ rx8 per 4-ktile group, one DMA) is the
    streamed rhs for all projections; weight tiles (w8 + rw8) are the
    stationary operand.
  - RoPE: rotate_half is two partition-shifted DVE multiplies against a
    sign-folded sin table; no PE work. Tables carry the 2^-15 descale.
  - scores are computed transposed (ST[k,q] = kT.T @ qT) for two heads
    at once in bf16; softmax skips max-subtraction; the denominator
    comes free as a ones-column appended to V.
  - partial-tile masks are host-computed 0/1 bf16 tiles applied
    multiplicatively after exp on GpSimd.
  - attention output (x16) is normalized per-partition, transposed on
    the PE in bf16, then DVE-quantized to e4m3 + residual head-pair
    tiles feeding the fp8 out-proj; partials bounce PSUM->SBUF with a
    2^-14 descale and stream to DRAM.
"""

import numpy as np
import ml_dtypes

import concourse.bass as bass
import concourse.tile as tile
from concourse import bacc, mybir
from concourse import bass_utils

# ---- problem constants -------------------------------------------------
T = 2048
HID = 4096
NUM_HEADS = 32
NUM_KV_HEADS = 8
D = 128  # head dim
WINDOW = 512
ROPE_THETA = 10000.0
N_CORES = 8
HPC = NUM_HEADS // N_CORES  # 4 q heads per core
QD = HPC * D  # 512 q-proj cols per core

NT = T // 128  # 16 token tiles
NKT = HID // 128  # 32 hidden k-tiles
NSTRIP = T // 512  # 4 token strips of 512
NOUT = HID // 512  # 8 output column slices

F32 = mybir.dt.float32
BF16 = mybir.dt.bfloat16
E4 = mybir.dt.float8e4
E4NP = ml_dtypes.float8_e4m3
SCALE = 1.0 / np.sqrt(D)
DR = mybir.MatmulPerfMode.DoubleRow

# fp8 quantization scales (powers of 2; residuals on the same grid)
SX = 32.0       # hidden_states
SW = 1024.0     # Wq/Wk/Wv
SA = 16.0       # attention output (folded into v descale)
SWO = 1024.0    # Wo
DESCALE_QK = 1.0 / (SX * SW)          # folded into cos/sin tables
DESCALE_V = SA / (SX * SW)            # applied at v PSUM->SBUF copy
DESCALE_O = 1.0 / (SA * SWO)          # applied at out-proj PSUM drain

_cache = {}


def _q8(x):
    """e4m3 quantize + same-grid residual (both as e4m3 arrays)."""
    b = x.astype(E4NP)
    r = (x - b.astype(np.float32)).astype(E4NP)
    return b, r


def _host_prep(cu_seqlens):
    """Everything derived from cu_seqlens: positions, rope tables,
    per-tile job list and mask tiles (ST layout [k, q], head-pair
    duplicated to [128, 256])."""
    cu = np.asarray(cu_seqlens, dtype=np.int64)
    tok = np.arange(T)
    seg = np.searchsorted(cu[1:], tok, side="right")
    pos = tok - cu[np.minimum(seg, len(cu) - 1)]

    same = seg[:, None] == seg[None, :]
    causal = pos[None, :] <= pos[:, None]
    win = pos[None, :] >= pos[:, None] - (WINDOW - 1)
    allowed = same & causal & win  # [q, k]

    jobs = []  # jobs[i] = [(j, mask_id | None), ...]
    masks = []
    mask_index = {}
    for i in range(NT):
        row = []
        for j in range(NT):
            blk = allowed[128 * i : 128 * (i + 1), 128 * j : 128 * (j + 1)]
            if not blk.any():
                continue
            if blk.all():
                row.append((j, None))
            else:
                key = blk.tobytes()
                if key not in mask_index:
                    mask_index[key] = len(masks)
                    masks.append(blk.T.astype(np.float32))  # ST layout
                row.append((j, mask_index[key]))
        jobs.append(row)
    if not masks:
        masks.append(np.ones((128, 128), np.float32))
    m = np.stack(masks)
    masks_np = np.concatenate([m, m], axis=2).astype(ml_dtypes.bfloat16)

    inv = 1.0 / (ROPE_THETA ** (np.arange(0, D, 2, dtype=np.float64) / D))
    freqs = pos[:, None].astype(np.float64) * inv[None, :]  # [T, 64]
    emb = np.concatenate([freqs, freqs], axis=1)  # [T, 128]
    # tables carry the fp8 descale for the q/k projections
    cos_t = (np.cos(emb).T * DESCALE_QK).astype(np.float32).copy()  # [128, T]
    sin_t = (np.sin(emb).T * DESCALE_QK).astype(np.float32)
    # sign-folded: rope(x)[d] = x[d]*cos[d] + x[(d+64)%128] * sin_s[d]
    sin_s = np.concatenate([-sin_t[:64], sin_t[64:]], axis=0).copy()
    ident = np.eye(128, dtype=ml_dtypes.bfloat16)

    return jobs, masks_np, cos_t, sin_s, ident


def _build(jobs, n_masks):
    """Trace the Bass/Tile program (identical on all cores)."""
    nc = bacc.Bacc("TRN2", target_bir_lowering=False, debug=False,
                   num_devices=N_CORES)

    # DRAM I/O (per-core shapes). ht carries x8 then rx8 for each
    # 4-ktile group: [strip, group, 128, base/resid, ktile, token]
    ht_d = nc.dram_tensor("ht", [NSTRIP, NKT // 4, 128, 2, 4, 512], E4,
                          kind="ExternalInput").ap()
    # weights: ktiles pre-grouped into DoubleRow kpairs [.., 16, 2, 128]
    wq_d = nc.dram_tensor("wq", [2, HPC, 128, NKT // 2, 2, 128], E4,
                          kind="ExternalInput").ap()  # [base/resid, h, ...]
    wk_d = nc.dram_tensor("wk", [2, 128, NKT // 2, 2, 128], E4,
                          kind="ExternalInput").ap()
    wv_d = nc.dram_tensor("wv", [2, 128, NKT // 2, 2, 128], E4,
                          kind="ExternalInput").ap()
    # wo: [base/resid, head-pair, d, head-in-pair, outcol]
    wo_d = nc.dram_tensor("wo", [2, 2, 128, 2, HID], E4,
                          kind="ExternalInput").ap()
    cos_d = nc.dram_tensor("cos_t", [128, T], F32, kind="ExternalInput").ap()
    sin_d = nc.dram_tensor("sin_s", [128, T], F32, kind="ExternalInput").ap()
    ident_d = nc.dram_tensor("ident", [128, 128], BF16,
                             kind="ExternalInput").ap()
    masks_d = nc.dram_tensor("masks", [n_masks, 128, 256], BF16,
                             kind="ExternalInput").ap()
    out_d = nc.dram_tensor("out", [T, HID], F32, kind="ExternalOutput").ap()

    with tile.TileContext(nc) as tc:
        with tc.tile_pool(name="persist", bufs=1) as pp:
            # resident weights / tables (base + residual, e4m3)
            wq_sb = [[pp.tile([128, NKT // 2, 2, 128], E4, name=f"wq{r}_{h}")
                      for h in range(HPC)] for r in range(2)]
            wk_sb = [pp.tile([128, NKT // 2, 2, 128], E4, name=f"wk{r}")
                     for r in range(2)]
            wv_sb = [pp.tile([128, NKT // 2, 2, 128], E4, name=f"wv{r}")
                     for r in range(2)]
            wo_sb = [[pp.tile([128, 2, HID], E4, name=f"wo{r}_{hp}")
                      for hp in range(2)] for r in range(2)]
            cos_sb = pp.tile([128, T], F32, name="cos_sb")
            sin_sb = pp.tile([128, T], F32, name="sin_sb")
            ident_sb = pp.tile([128, 128], BF16, name="ident_sb")
            mask_sb = [pp.tile([128, 256], BF16, name=f"mask{m}")
                       for m in range(n_masks)]
            # activations produced by phase 1, consumed by phase 2
            # qt pairs: [128, 2*T]; cols [256*i + 128*m : +128] = head
            # (2*hp + m), token tile i.
            qt_sb = [pp.tile([128, 2 * T], BF16, name=f"qtp{hp}")
                     for hp in range(2)]
            kt_sb = pp.tile([128, T], BF16, name="kt_sb")
            vaug_sb = [pp.tile([128, D + 1], BF16, name=f"vaug{t}")
                       for t in range(NT)]

            qt_4d = [q.rearrange("p (i m c) -> p i m c", m=2, c=128)
                     for q in qt_sb]

            for t in range(NT):
                nc.vector.memset(vaug_sb[t][:, D : D + 1], 1.0)

            # ---------------- phase 1: projections + RoPE ----------------
            with (
                tc.tile_pool(name="ht_pool", bufs=6) as htp,
                tc.tile_pool(name="rope_tmp", bufs=4) as rtp,
                tc.tile_pool(name="proj_psum", bufs=6, space="PSUM") as ppp,
                tc.tile_pool(name="util_psum", bufs=2, space="PSUM") as upp,
            ):
                def rope(s, h, src):
                    """src: fp32 PSUM [128, 512] pre-rope projection
                    (carries SX*SW scale; tables descale it)."""
                    ssl = bass.ts(s, 512)
                    if h < HPC:
                        dst = qt_4d[h // 2][:, 4 * s : 4 * s + 4, h % 2, :]
                    else:
                        dst = kt_sb[:, ssl]
                    raw = rtp.tile([128, 512], F32, tag="raw",
                                   name=f"raw{s}_{h}")
                    nc.scalar.copy(raw[:], src[:])
                    t1 = rtp.tile([128, 512], F32, tag="t1",
                                  name=f"t1_{s}_{h}")
                    nc.gpsimd.tensor_mul(t1[:], raw[:], cos_sb[:, ssl])
                    # rotate_half: walrus requires TT operands to share a
                    # start partition, so swap halves via copies first
                    # (partition-shifted copies are legal; signs live in sin_s)
                    sw = rtp.tile([128, 512], F32, tag="sw",
                                  name=f"sw{s}_{h}")
                    nc.vector.tensor_scalar_mul(sw[0:64, :],
                                                raw[64:128, :], 1.0)
                    nc.vector.tensor_scalar_mul(sw[64:128, :],
                                                raw[0:64, :], 1.0)
                    t2 = rtp.tile([128, 512], F32, tag="t2",
                                  name=f"t2_{s}_{h}")
                    nc.vector.tensor_mul(t2[:], sw[:], sin_sb[:, ssl])
                    if h < HPC:
                        t1v = t1.rearrange("p (i c) -> p i c", c=128)
                        t2v = t2.rearrange("p (i c) -> p i c", c=128)
                    else:
                        t1v, t2v = t1[:], t2[:]
                    nc.vector.tensor_add(dst, t1v, t2v)

                def v_pipeline(s, ps_v):
                    """ps_v: vT strip PSUM -> 4 v_aug tiles [k, dim].
                    Applies the fp8 descale (and the SA pre-scale)."""
                    vts = rtp.tile([128, 512], BF16, tag="vts", name=f"vts{s}")
                    nc.vector.tensor_scalar_mul(vts[:], ps_v[:],
                                                float(DESCALE_V))
                    vtp = upp.tile([128, 512], BF16, tag="util", name=f"vtp{s}")
                    for tt in range(4):
                        tsl = bass.ts(tt, 128)
                        nc.tensor.transpose(vtp[:, tsl], vts[:, tsl],
                                            ident_sb[:])
                        nc.vector.tensor_copy(vaug_sb[4 * s + tt][:, 0:D],
                                              vtp[:, tsl])

                def proj_round(s, heads, preamble=None, postamble=None):
                    """One k-loop computing fp8 3-term projections `heads`
                    (0..3 = q, 4 = k, 5 = v) for strip s into len(heads)
                    PSUM banks via DoubleRow matmuls."""
                    ps = [ppp.tile([128, 512], F32, tag="proj",
                                   name=f"ps{s}_{h}") for h in heads]
                    wbv = {4: wk_sb, 5: wv_sb}
                    for g in range(NKT // 4):
                        if preamble is not None:
                            preamble(g)
                        # one DMA carries x8 + rx8 for 4 hidden k-tiles
                        ht_t = htp.tile([128, 2, 4, 512], E4, tag="ht",
                                        name=f"ht{s}_{g}_{heads[0]}")
                        nc.sync.dma_start(ht_t[:], ht_d[s, g])
                        if postamble is not None:
                            postamble(g)
                        for kp in range(2):  # kpair within group
                            gk = 2 * g + kp  # global kpair index
                            first_k = gk == 0
                            last_k = gk == NKT // 2 - 1
                            for ps_t, h in zip(ps, heads):
                                w8 = (wq_sb[0][h] if h < HPC
                                      else wbv[h][0])[:, gk]
                                rw8 = (wq_sb[1][h] if h < HPC
                                       else wbv[h][1])[:, gk]
                                for th in range(2):  # token half
                                    rhs_x = ht_t[:, 0, 2 * kp : 2 * kp + 2,
                                                 bass.ts(th, 256)]
                                    rhs_r = ht_t[:, 1, 2 * kp : 2 * kp + 2,
                                                 bass.ts(th, 256)]
                                    osl = ps_t[:, bass.ts(th, 256)]
                                    start = first_k and th == 0
                                    stop = last_k and th == 1
                                    nc.tensor.matmul(osl, w8, rhs_x,
                                                     start=start, stop=False,
                                                     perf_mode=DR)
                                    nc.tensor.matmul(osl, rw8, rhs_x,
                                                     start=False, stop=False,
                                                     perf_mode=DR)
                                    nc.tensor.matmul(osl, w8, rhs_r,
                                                     start=False, stop=stop,
                                                     perf_mode=DR)
                    return ps

                def strip0_preamble(g):
                    # the very first matmuls only need wq0's chunk; the rest
                    # of each weight-chunk group queues behind the ht tile
                    ksl = bass.ds(2 * g, 2)
                    nc.sync.dma_start(wq_sb[0][0][:, ksl], wq_d[0, 0][:, ksl])

                def strip0_postamble(g):
                    ksl = bass.ds(2 * g, 2)
                    nc.sync.dma_start(wq_sb[1][0][:, ksl], wq_d[1, 0][:, ksl])
                    for h in range(1, HPC):
                        for r in range(2):
                            nc.sync.dma_start(wq_sb[r][h][:, ksl],
                                              wq_d[r, h][:, ksl])
                    for r in range(2):
                        nc.sync.dma_start(wk_sb[r][:, ksl], wk_d[r][:, ksl])
                        nc.sync.dma_start(wv_sb[r][:, ksl], wv_d[r][:, ksl])

                def table_chunk(s):
                    # rope-table chunk for strip s, just before its RoPE
                    csl = bass.ts(s, 512)
                    nc.sync.dma_start(cos_sb[:, csl], cos_d[:, csl])
                    nc.sync.dma_start(sin_sb[:, csl], sin_d[:, csl])
                    if s == 0:
                        nc.sync.dma_start(ident_sb[:], ident_d[:])
                    if s == 1:
                        for m in range(n_masks):
                            nc.sync.dma_start(mask_sb[m][:], masks_d[m])

                def wo_chunk(s):
                    # wo is only needed in phase 2; trickle one of the four
                    # [128, 2, HID] e4m3 tiles per strip
                    r, hp = divmod(s, 2)
                    nc.sync.dma_start(wo_sb[r][hp][:], wo_d[r, hp])

                for s in range(NSTRIP - 1):
                    ps = proj_round(s, [0, 1, 2, 3, 4, 5],
                                    preamble=strip0_preamble if s == 0 else None,
                                    postamble=strip0_postamble if s == 0 else None)
                    table_chunk(s)
                    if s >= 1:
                        wo_chunk(s - 1)
                    v_pipeline(s, ps[5])
                    for h in range(HPC + 1):
                        rope(s, h, ps[h])

                # Last strip in two 3-bank rounds (hT re-streamed): round A's
                # banks drain during round B's matmuls, so phase 2's PSUM
                # pools don't stall on the phase-1 epilogue.
                s = NSTRIP - 1
                ps_a = proj_round(s, [0, 1, 4])
                table_chunk(s)
                wo_chunk(s - 1)
                for h in (0, 1, 4):
                    rope(s, h, ps_a[(0, 1, 4).index(h)])
                ps_b = proj_round(s, [5, 2, 3])
                wo_chunk(s)
                v_pipeline(s, ps_b[0])
                for h in (2, 3):
                    rope(s, h, ps_b[(5, 2, 3).index(h)])

            # ---------------- phase 2: attention + out proj --------------
            with (
                tc.tile_pool(name="attn_sbuf", bufs=8) as asp,
                tc.tile_pool(name="attn_small", bufs=4) as asmall,
                tc.tile_pool(name="score_psum", bufs=3, space="PSUM") as spp,
                tc.tile_pool(name="oaug_psum", bufs=2, space="PSUM") as opp,
                tc.tile_pool(name="oproj_psum", bufs=3, space="PSUM") as prp,
            ):

                def oproj(i, at8_list, rat8_list):
                    """fp8 3-term out-proj for token tile i. at8/rat8:
                    per head-pair [128, 2, 128] e4m3 (x SA)."""
                    isl = bass.ts(i, 128)
                    for ns in range(NOUT):
                        po = prp.tile([128, 512], F32, tag="oproj",
                                      name=f"po{i}_{ns}")
                        n_in = 12
                        n = 0
                        for hp in range(2):
                            for th in range(2):
                                osl = po[:, bass.ts(th, 256)]
                                csl = bass.ds(512 * ns + 256 * th, 256)
                                w8 = wo_sb[0][hp][:, :, csl]
                                rw8 = wo_sb[1][hp][:, :, csl]
                                a8 = at8_list[hp][:]
                                ra8 = rat8_list[hp][:]
                                for lhs, rhs in ((a8, w8), (ra8, w8),
                                                 (a8, rw8)):
                                    nc.tensor.matmul(
                                        osl, lhs, rhs,
                                        start=(n == 0), stop=(n == n_in - 1),
                                        perf_mode=DR)
                                    n += 1
                        po_sb = asp.tile([128, 512], F32, tag="posb", bufs=4,
                                         name=f"posb{i}_{ns}")
                        if ns % 2 == 0:
                            nc.vector.tensor_scalar_mul(po_sb[:], po[:],
                                                        float(DESCALE_O))
                        else:
                            nc.scalar.mul(po_sb[:], po[:], float(DESCALE_O))
                        nc.sync.dma_start(out_d[isl, bass.ts(ns, 512)],
                                          po_sb[:])

                prev_at = None
                for i in range(NT):
                    at8_pair = [asp.tile([128, 2, 128], E4, tag="at8",
                                         bufs=5, name=f"at8_{i}_{hp}")
                                for hp in range(2)]
                    rat8_pair = [asp.tile([128, 2, 128], E4, tag="rat8",
                                          bufs=5, name=f"rat8_{i}_{hp}")
                                 for hp in range(2)]
                    njobs = len(jobs[i])
                    for hp in range(2):
                        ps_o = [opp.tile([128, D + 1], F32, tag="oaug",
                                         name=f"pso{i}_{2 * hp + m}")
                                for m in range(2)]
                        # j-tiles in pairs: two score matmuls fill one
                        # [128,512] PSUM bank, one exp covers both, then the
                        # four PV matmuls consume quarter slices
                        jl = jobs[i]
                        for p0 in range(0, njobs, 2):
                            pair = jl[p0 : p0 + 2]
                            w = 256 * len(pair)
                            ps_s = spp.tile([128, 512], F32, tag="score",
                                            name=f"pss{i}_{hp}_{p0}")
                            for q, (j, mid) in enumerate(pair):
                                nc.tensor.matmul(
                                    ps_s[:, bass.ts(q, 256)],
                                    kt_sb[:, bass.ts(j, 128)],
                                    qt_sb[hp][:, bass.ts(i, 256)],
                                    start=True, stop=True)
                            se = asp.tile([128, 512], BF16, tag="sexp",
                                          name=f"se{i}_{hp}_{p0}")
                            nc.scalar.activation(
                                se[:, 0:w], ps_s[:, 0:w],
                                mybir.ActivationFunctionType.Exp,
                                bias=0.0, scale=float(SCALE))
                            for q, (j, mid) in enumerate(pair):
                                if mid is not None:
                                    nc.gpsimd.tensor_mul(
                                        se[:, bass.ts(q, 256)],
                                        se[:, bass.ts(q, 256)],
                                        mask_sb[mid][:])
                            for q, (j, mid) in enumerate(pair):
                                jj = p0 + q
                                for m in range(2):
                                    nc.tensor.matmul(
                                        ps_o[m][:],
                                        se[:, bass.ds(256 * q + 128 * m, 128)],
                                        vaug_sb[j][:],
                                        start=(jj == 0),
                                        stop=(jj == njobs - 1))
                        for m in range(2):
                            h = 2 * hp + m
                            recip = asmall.tile([128, 1], F32, tag="recip",
                                                name=f"rc{i}_{h}")
                            nc.vector.reciprocal(recip[:],
                                                 ps_o[m][:, D : D + 1])
                            a_n = asp.tile([128, 128], BF16, tag="anorm",
                                           name=f"an{i}_{h}")
                            nc.vector.tensor_scalar_mul(a_n[:],
                                                        ps_o[m][:, 0:D],
                                                        recip[:])
                            at_p = spp.tile([128, 128], BF16, tag="score",
                                            name=f"atp{i}_{h}")
                            nc.tensor.transpose(at_p[:], a_n[:], ident_sb[:])
                            at = asp.tile([128, 128], BF16, tag="at",
                                          bufs=10, name=f"at{i}_{h}")
                            nc.vector.tensor_copy(at[:], at_p[:])
                            # quantize to e4m3 + residual (values carry SA)
                            nc.vector.tensor_copy(at8_pair[hp][:, m], at[:])
                            nc.vector.tensor_sub(rat8_pair[hp][:, m], at[:],
                                                 at8_pair[hp][:, m])

                    if prev_at is not None:
                        oproj(i - 1, *prev_at)
                    prev_at = (at8_pair, rat8_pair)
                oproj(NT - 1, *prev_at)

    nc.compile()
    return nc


def _get_nc(cu_seqlens):
    key = np.asarray(cu_seqlens).tobytes()
    if key not in _cache:
        jobs, masks_np, cos_t, sin_s, ident = _host_prep(cu_seqlens)
        nc = _build(jobs, masks_np.shape[0])
        _cache[key] = (nc, masks_np, cos_t, sin_s, ident)
    return _cache[key]


def kernel(hidden_states, Wq, Wk, Wv, Wo, cu_seqlens):
    hidden_states = np.asarray(hidden_states)
    Wq, Wk, Wv, Wo = (np.asarray(a) for a in (Wq, Wk, Wv, Wo))
    cu_seqlens = np.asarray(cu_seqlens)
    nc, masks_np, cos_t, sin_s, ident = _get_nc(cu_seqlens)

    # hT in fp8: base + residual at scale SX
    ht = np.ascontiguousarray(hidden_states.T) * np.float32(SX)
    h8, hr8 = _q8(ht)
    # tile for contiguous DMA: [NSTRIP, NKT//4, 128, 2, 4, 512] — each
    # DMA carries 4 hidden k-tiles of x8 then the matching rx8
    def tile_ht(a):
        return np.ascontiguousarray(
            a.reshape(NKT // 4, 4, 128, NSTRIP, 512).transpose(3, 0, 2, 1, 4)
        ).reshape(NSTRIP, NKT // 4, 128, 1, 4, 512)
    ht_tiled = np.concatenate([tile_ht(h8), tile_ht(hr8)], axis=3)

    in_maps = []
    for c in range(N_CORES):
        wq_c = Wq[:, QD * c : QD * (c + 1)].astype(np.float32) * np.float32(SW)
        wq8, wqr = _q8(wq_c)
        # [2, HPC, 128, 16, 2, 128]: lhsT tiles, ktiles grouped in kpairs
        def tile_wq(a):
            return np.ascontiguousarray(
                a.reshape(NKT, 128, HPC, 128).transpose(2, 1, 0, 3)
            ).reshape(HPC, 128, NKT // 2, 2, 128)
        wq_t = np.stack([tile_wq(wq8), tile_wq(wqr)])
        def tile_wkv(a):
            return np.ascontiguousarray(
                a.reshape(NKT, 128, 128).transpose(1, 0, 2)
            ).reshape(128, NKT // 2, 2, 128)
        wk8, wkr = _q8(Wk[:, D * c : D * (c + 1)].astype(np.float32)
                       * np.float32(SW))
        wk_t = np.stack([tile_wkv(wk8), tile_wkv(wkr)])
        wv8, wvr = _q8(Wv[:, D * c : D * (c + 1)].astype(np.float32)
                       * np.float32(SW))
        wv_t = np.stack([tile_wkv(wv8), tile_wkv(wvr)])
        # wo: [2, hpair, 128 d, 2 head-in-pair, HID]
        wo_c = Wo[QD * c : QD * (c + 1), :].astype(np.float32) \
            * np.float32(SWO)
        wo8, wor = _q8(wo_c)
        def tile_wo(a):
            return np.ascontiguousarray(
                a.reshape(2, 2, 128, HID).transpose(0, 2, 1, 3))
        wo_t = np.stack([tile_wo(wo8), tile_wo(wor)])
        in_maps.append({
            "ht": ht_tiled, "wq": wq_t, "wk": wk_t, "wv": wv_t, "wo": wo_t,
            "cos_t": cos_t, "sin_s": sin_s, "ident": ident,
            "masks": masks_np,
        })

    res = bass_utils.run_bass_kernel_spmd(nc, in_maps,
                                          core_ids=list(range(N_CORES)))
    out = res.results[0]["out"].astype(np.float64)
    for c in range(1, N_CORES):
        out += res.results[c]["out"]
    return out.astype(np.float32)


# revision 20
# speedup vs baseline: 1.1909x; 1.1218x over previous
"""Trainium2 Bass kernel for Mistral-style sliding-window GQA attention.

Problem (hardcoded shapes):
  hidden_states [2048, 4096] f32, Wq [4096, 4096], Wk/Wv [4096, 1024],
  Wo [4096, 4096], cu_seqlens [3] int32. 32 Q heads / 8 KV heads,
  head_dim 128, sliding window 512, rope theta 10000.

Sharding: tensor-parallel over heads across 8 cores. Core c owns Q heads
[4c, 4c+4) and KV head c (GQA groups align: qh//4 == c). Wq/Wk/Wv are
column-sharded, Wo row-sharded; each core emits a partial [2048, 4096]
output which the host sums.

Device kernel layout choices (per core):
  - The four big GEMMs (q/k/v projections, out-proj) run in fp8 e4m3
    DoubleRow mode (2 k-tiles per PE instruction at 0.5 cycles/row) with
    a 3-term residual decomposition X@W ~= X8@W8 + RX8@W8 + X8@RW8.
    Operands are quantized at power-of-2 scales (X*32, W*1024, A*16)
    with residuals on the same grid, so all three terms accumulate in
    one PSUM bank and the single descale folds into existing table /
    copy steps (cos/sin tables, v copy, out-proj PSUM drain). Each bank
    fill has exactly ONE start=True (first instr) and ONE stop=True
    (last instr): start marks the whole 2KB bank pending-zero, so each
    column-half's first write lands on zeroed bytes.
  - hT = hidden^T in e4m3 (x8 ++ rx8 per 4-ktile group, one DMA) is the
    streamed rhs for all projections; weight tiles (w8 + rw8) are the
    stationary operand.
  - RoPE: rotate_half is two partition-shifted DVE multiplies against a
    sign-folded sin table; no PE work. Tables carry the 2^-15 descale.
  - scores are computed transposed (ST[k,q] = kT.T @ qT) for two heads
    at once in bf16; softmax skips max-subtraction; the denominator
    comes free as a ones-column appended to V.
  - partial-tile masks are host-computed 0/1 bf16 tiles applied
    multiplicatively after exp on GpSimd.
  - attention output (x16) is normalized per-partition, transposed on
    the PE in bf16, then DVE-quantized to e4m3 + residual head-pair
    tiles feeding the fp8 out-proj; partials bounce PSUM->SBUF with a
    2^-14 descale and stream to DRAM.
"""

import numpy as np
import ml_dtypes

import concourse.bass as bass
import concourse.tile as tile
from concourse import bacc, mybir
from concourse import bass_utils

# ---- problem constants -------------------------------------------------
T = 2048
HID = 4096
NUM_HEADS = 32
NUM_KV_HEADS = 8
D = 128  # head dim
WINDOW = 512
ROPE_THETA = 10000.0
N_CORES = 8
HPC = NUM_HEADS // N_CORES  # 4 q heads per core
QD = HPC * D  # 512 q-proj cols per core

NT = T // 128  # 16 token tiles
NKT = HID // 128  # 32 hidden k-tiles
NSTRIP = T // 512  # 4 token strips of 512
NOUT = HID // 512  # 8 output column slices

F32 = mybir.dt.float32
BF16 = mybir.dt.bfloat16
E4 = mybir.dt.float8e4
E4NP = ml_dtypes.float8_e4m3
SCALE = 1.0 / np.sqrt(D)
DR = mybir.MatmulPerfMode.DoubleRow

# fp8 quantization scales (powers of 2; residuals on the same grid)
SX = 32.0       # hidden_states
SW = 1024.0     # Wq/Wk/Wv
SA = 16.0       # attention output (folded into v descale)
SWO = 1024.0    # Wo
DESCALE_QK = 1.0 / (SX * SW)          # folded into cos/sin tables
DESCALE_V = SA / (SX * SW)            # applied at v PSUM->SBUF copy
DESCALE_O = 1.0 / (SA * SWO)          # applied at out-proj PSUM drain

_cache = {}


def _q8(x):
    """e4m3 quantize + same-grid residual (both as e4m3 arrays)."""
    b = x.astype(E4NP)
    r = (x - b.astype(np.float32)).astype(E4NP)
    return b, r


def _host_prep(cu_seqlens):
    """Everything derived from cu_seqlens: positions, rope tables,
    per-tile job list and mask tiles (ST layout [k, q], head-pair
    duplicated to [128, 256])."""
    cu = np.asarray(cu_seqlens, dtype=np.int64)
    tok = np.arange(T)
    seg = np.searchsorted(cu[1:], tok, side="right")
    pos = tok - cu[np.minimum(seg, len(cu) - 1)]

    same = seg[:, None] == seg[None, :]
    causal = pos[None, :] <= pos[:, None]
    win = pos[None, :] >= pos[:, None] - (WINDOW - 1)
    allowed = same & causal & win  # [q, k]

    jobs = []  # jobs[i] = [(j, mask_id | None), ...]
    masks = []
    mask_index = {}
    for i in range(NT):
        row = []
        for j in range(NT):
            blk = allowed[128 * i : 128 * (i + 1), 128 * j : 128 * (j + 1)]
            if not blk.any():
                continue
            if blk.all():
                row.append((j, None))
            else:
                key = blk.tobytes()
                if key not in mask_index:
                    mask_index[key] = len(masks)
                    masks.append(blk.T.astype(np.float32))  # ST layout
                row.append((j, mask_index[key]))
        jobs.append(row)
    if not masks:
        masks.append(np.ones((128, 128), np.float32))
    m = np.stack(masks)
    m = np.concatenate([m, m], axis=2).astype(ml_dtypes.bfloat16)
    # single [128, n_masks*256] tensor for one batched DMA
    masks_np = np.ascontiguousarray(
        m.transpose(1, 0, 2).reshape(128, -1))

    inv = 1.0 / (ROPE_THETA ** (np.arange(0, D, 2, dtype=np.float64) / D))
    freqs = pos[:, None].astype(np.float64) * inv[None, :]  # [T, 64]
    emb = np.concatenate([freqs, freqs], axis=1)  # [T, 128]
    # tables carry the fp8 descale for the q/k projections
    cos_t = (np.cos(emb).T * DESCALE_QK).astype(np.float32).copy()  # [128, T]
    sin_t = (np.sin(emb).T * DESCALE_QK).astype(np.float32)
    # sign-folded: rope(x)[d] = x[d]*cos[d] + x[(d+64)%128] * sin_s[d]
    sin_s = np.concatenate([-sin_t[:64], sin_t[64:]], axis=0).copy()
    ident = np.eye(128, dtype=ml_dtypes.bfloat16)

    return jobs, masks_np, cos_t, sin_s, ident


def _build(jobs, n_masks):
    """Trace the Bass/Tile program (identical on all cores)."""
    nc = bacc.Bacc("TRN2", target_bir_lowering=False, debug=False,
                   num_devices=N_CORES)

    # DRAM I/O (per-core shapes). ht carries x8 then rx8 for each
    # 4-ktile group: [strip, group, 128, base/resid, ktile, token]
    ht_d = nc.dram_tensor("ht", [NSTRIP, NKT // 4, 128, 2, 4, 512], E4,
                          kind="ExternalInput").ap()
    # all 12 qkv weight tensors (4 wq + wk + wv, base then resid),
    # group-interleaved so strip 0 loads one 6KB-row DMA per k-group:
    # [group, 128, tensor, kpair-in-group, sub, 128]
    wall_d = nc.dram_tensor("wall", [NKT // 4, 128, 12, 2, 2, 128], E4,
                            kind="ExternalInput").ap()
    # wo: [base/resid, head-pair, d, head-in-pair, outcol]
    wo_d = nc.dram_tensor("wo", [2, 2, 128, 2, HID], E4,
                          kind="ExternalInput").ap()
    cos_d = nc.dram_tensor("cos_t", [128, T], F32, kind="ExternalInput").ap()
    sin_d = nc.dram_tensor("sin_s", [128, T], F32, kind="ExternalInput").ap()
    ident_d = nc.dram_tensor("ident", [128, 128], BF16,
                             kind="ExternalInput").ap()
    masks_d = nc.dram_tensor("masks", [128, n_masks * 256], BF16,
                             kind="ExternalInput").ap()
    out_d = nc.dram_tensor("out", [T, HID], BF16, kind="ExternalOutput").ap()

    with tile.TileContext(nc) as tc:
        with tc.tile_pool(name="persist", bufs=1) as pp:
            # resident weights / tables (base + residual, e4m3)
            # w_all: [128, group, tensor(12), kpair-in-group, sub, 128];
            # tensor order: wq base h0-3, wk base, wv base, then residuals
            w_all = pp.tile([128, NKT // 4, 12, 2, 2, 128], E4, name="w_all")
            wo_sb = [[pp.tile([128, 2, HID], E4, name=f"wo{r}_{hp}")
                      for hp in range(2)] for r in range(2)]
            cos_sb = pp.tile([128, T], F32, name="cos_sb")
            sin_sb = pp.tile([128, T], F32, name="sin_sb")
            ident_sb = pp.tile([128, 128], BF16, name="ident_sb")
            mask_sb = pp.tile([128, n_masks * 256], BF16, name="mask_sb")
            # activations produced by phase 1, consumed by phase 2
            # qt pairs: [128, 2*T]; cols [256*i + 128*m : +128] = head
            # (2*hp + m), token tile i.
            qt_sb = [pp.tile([128, 2 * T], BF16, name=f"qtp{hp}")
                     for hp in range(2)]
            kt_sb = pp.tile([128, T], BF16, name="kt_sb")
            vaug_sb = [pp.tile([128, D + 1], BF16, name=f"vaug{t}")
                       for t in range(NT)]

            qt_4d = [q.rearrange("p (i m c) -> p i m c", m=2, c=128)
                     for q in qt_sb]

            for t in range(NT):
                nc.vector.memset(vaug_sb[t][:, D : D + 1], 1.0)

            # ---------------- phase 1: projections + RoPE ----------------
            with (
                tc.tile_pool(name="ht_pool", bufs=6) as htp,
                tc.tile_pool(name="rope_tmp", bufs=4) as rtp,
                tc.tile_pool(name="proj_psum", bufs=6, space="PSUM") as ppp,
                tc.tile_pool(name="util_psum", bufs=2, space="PSUM") as upp,
            ):
                def rope(s, h, src):
                    """src: fp32 PSUM [128, 512] pre-rope projection
                    (carries SX*SW scale; tables descale it)."""
                    ssl = bass.ts(s, 512)
                    if h < HPC:
                        dst = qt_4d[h // 2][:, 4 * s : 4 * s + 4, h % 2, :]
                    else:
                        dst = kt_sb[:, ssl]
                    raw = rtp.tile([128, 512], F32, tag="raw",
                                   name=f"raw{s}_{h}")
                    nc.scalar.copy(raw[:], src[:])
                    t1 = rtp.tile([128, 512], F32, tag="t1",
                                  name=f"t1_{s}_{h}")
                    nc.gpsimd.tensor_mul(t1[:], raw[:], cos_sb[:, ssl])
                    # rotate_half: walrus requires TT operands to share a
                    # start partition, so swap halves via copies first
                    # (partition-shifted copies are legal; signs live in sin_s)
                    sw = rtp.tile([128, 512], F32, tag="sw",
                                  name=f"sw{s}_{h}")
                    nc.vector.tensor_scalar_mul(sw[0:64, :],
                                                raw[64:128, :], 1.0)
                    nc.vector.tensor_scalar_mul(sw[64:128, :],
                                                raw[0:64, :], 1.0)
                    t2 = rtp.tile([128, 512], F32, tag="t2",
                                  name=f"t2_{s}_{h}")
                    nc.vector.tensor_mul(t2[:], sw[:], sin_sb[:, ssl])
                    if h < HPC:
                        t1v = t1.rearrange("p (i c) -> p i c", c=128)
                        t2v = t2.rearrange("p (i c) -> p i c", c=128)
                    else:
                        t1v, t2v = t1[:], t2[:]
                    nc.vector.tensor_add(dst, t1v, t2v)

                def v_pipeline(s, ps_v):
                    """ps_v: vT strip PSUM -> 4 v_aug tiles [k, dim].
                    Applies the fp8 descale (and the SA pre-scale)."""
                    vts = rtp.tile([128, 512], BF16, tag="vts", name=f"vts{s}")
                    nc.vector.tensor_scalar_mul(vts[:], ps_v[:],
                                                float(DESCALE_V))
                    vtp = upp.tile([128, 512], BF16, tag="util", name=f"vtp{s}")
                    for tt in range(4):
                        tsl = bass.ts(tt, 128)
                        nc.tensor.transpose(vtp[:, tsl], vts[:, tsl],
                                            ident_sb[:])
                        nc.vector.tensor_copy(vaug_sb[4 * s + tt][:, 0:D],
                                              vtp[:, tsl])

                def proj_round(s, heads, preamble=None):
                    """One k-loop computing fp8 3-term projections `heads`
                    (0..3 = q, 4 = k, 5 = v) for strip s into len(heads)
                    PSUM banks via DoubleRow matmuls."""
                    ps = [ppp.tile([128, 512], F32, tag="proj",
                                   name=f"ps{s}_{h}") for h in heads]
                    for g in range(NKT // 4):
                        if preamble is not None:
                            preamble(g)
                        # one DMA carries x8 + rx8 for 4 hidden k-tiles
                        ht_t = htp.tile([128, 2, 4, 512], E4, tag="ht",
                                        name=f"ht{s}_{g}_{heads[0]}")
                        nc.sync.dma_start(ht_t[:], ht_d[s, g])
                        for kp in range(2):  # kpair within group
                            gk = 2 * g + kp  # global kpair index
                            first_k = gk == 0
                            last_k = gk == NKT // 2 - 1
                            for ps_t, h in zip(ps, heads):
                                w8 = w_all[:, g, h, kp]
                                rw8 = w_all[:, g, h + 6, kp]
                                for th in range(2):  # token half
                                    rhs_x = ht_t[:, 0, 2 * kp : 2 * kp + 2,
                                                 bass.ts(th, 256)]
                                    rhs_r = ht_t[:, 1, 2 * kp : 2 * kp + 2,
                                                 bass.ts(th, 256)]
                                    osl = ps_t[:, bass.ts(th, 256)]
                                    start = first_k and th == 0
                                    stop = last_k and th == 1
                                    nc.tensor.matmul(osl, w8, rhs_x,
                                                     start=start, stop=False,
                                                     perf_mode=DR)
                                    nc.tensor.matmul(osl, rw8, rhs_x,
                                                     start=False, stop=False,
                                                     perf_mode=DR)
                                    nc.tensor.matmul(osl, w8, rhs_r,
                                                     start=False, stop=stop,
                                                     perf_mode=DR)
                    return ps

                def strip0_preamble(g):
                    # one 6KB-row DMA carries every tensor's chunk for group g
                    nc.sync.dma_start(w_all[:, g], wall_d[g])

                def table_chunk(s):
                    # rope-table chunk for strip s, just before its RoPE
                    csl = bass.ts(s, 512)
                    nc.sync.dma_start(cos_sb[:, csl], cos_d[:, csl])
                    nc.sync.dma_start(sin_sb[:, csl], sin_d[:, csl])
                    if s == 0:
                        nc.sync.dma_start(ident_sb[:], ident_d[:])
                    if s == 1:
                        nc.sync.dma_start(mask_sb[:], masks_d)

                def wo_chunk(s):
                    # wo is only needed in phase 2; trickle one of the four
                    # [128, 2, HID] e4m3 tiles per strip
                    r, hp = divmod(s, 2)
                    nc.sync.dma_start(wo_sb[r][hp][:], wo_d[r, hp])

                for s in range(NSTRIP - 1):
                    ps = proj_round(s, [0, 1, 2, 3, 4, 5],
                                    preamble=strip0_preamble if s == 0 else None)
                    table_chunk(s)
                    if s >= 1:
                        wo_chunk(s - 1)
                    v_pipeline(s, ps[5])
                    for h in range(HPC + 1):
                        rope(s, h, ps[h])

                # Last strip in two 3-bank rounds (hT re-streamed): round A's
                # banks drain during round B's matmuls, so phase 2's PSUM
                # pools don't stall on the phase-1 epilogue.
                s = NSTRIP - 1
                ps_a = proj_round(s, [0, 1, 4])
                table_chunk(s)
                wo_chunk(s - 1)
                for h in (0, 1, 4):
                    rope(s, h, ps_a[(0, 1, 4).index(h)])
                ps_b = proj_round(s, [5, 2, 3])
                wo_chunk(s)
                v_pipeline(s, ps_b[0])
                for h in (2, 3):
                    rope(s, h, ps_b[(5, 2, 3).index(h)])

            # ---------------- phase 2: attention + out proj --------------
            with (
                tc.tile_pool(name="attn_sbuf", bufs=8) as asp,
                tc.tile_pool(name="attn_small", bufs=4) as asmall,
                tc.tile_pool(name="score_psum", bufs=3, space="PSUM") as spp,
                tc.tile_pool(name="oaug_psum", bufs=2, space="PSUM") as opp,
                tc.tile_pool(name="oproj_psum", bufs=3, space="PSUM") as prp,
            ):

                def oproj(i, at8_list, rat8_list):
                    """fp8 3-term out-proj for token tile i. at8/rat8:
                    per head-pair [128, 2, 128] e4m3 (x SA)."""
                    isl = bass.ts(i, 128)
                    otile = asp.tile([128, HID], BF16, tag="obat", bufs=2,
                                     name=f"ob{i}")
                    for ns in range(NOUT):
                        po = prp.tile([128, 512], F32, tag="oproj",
                                      name=f"po{i}_{ns}")
                        n_in = 12
                        n = 0
                        for hp in range(2):
                            for th in range(2):
                                osl = po[:, bass.ts(th, 256)]
                                csl = bass.ds(512 * ns + 256 * th, 256)
                                w8 = wo_sb[0][hp][:, :, csl]
                                rw8 = wo_sb[1][hp][:, :, csl]
                                a8 = at8_list[hp][:]
                                ra8 = rat8_list[hp][:]
                                for lhs, rhs in ((a8, w8), (ra8, w8),
                                                 (a8, rw8)):
                                    nc.tensor.matmul(
                                        osl, lhs, rhs,
                                        start=(n == 0), stop=(n == n_in - 1),
                                        perf_mode=DR)
                                    n += 1
                        osb = otile[:, bass.ts(ns, 512)]
                        if ns % 2 == 0:
                            nc.vector.tensor_scalar_mul(osb, po[:],
                                                        float(DESCALE_O))
                        else:
                            nc.scalar.mul(osb, po[:], float(DESCALE_O))
                    nc.sync.dma_start(out_d[isl], otile[:])

                prev_at = None
                for i in range(NT):
                    at8_pair = [asp.tile([128, 2, 128], E4, tag="at8",
                                         bufs=5, name=f"at8_{i}_{hp}")
                                for hp in range(2)]
                    rat8_pair = [asp.tile([128, 2, 128], E4, tag="rat8",
                                          bufs=5, name=f"rat8_{i}_{hp}")
                                 for hp in range(2)]
                    njobs = len(jobs[i])
                    for hp in range(2):
                        ps_o = [opp.tile([128, D + 1], F32, tag="oaug",
                                         name=f"pso{i}_{2 * hp + m}")
                                for m in range(2)]
                        # j-tiles in pairs: two score matmuls fill one
                        # [128,512] PSUM bank, one exp covers both, then the
                        # four PV matmuls consume quarter slices
                        jl = jobs[i]
                        for p0 in range(0, njobs, 2):
                            pair = jl[p0 : p0 + 2]
                            w = 256 * len(pair)
                            ps_s = spp.tile([128, 512], F32, tag="score",
                                            name=f"pss{i}_{hp}_{p0}")
                            for q, (j, mid) in enumerate(pair):
                                nc.tensor.matmul(
                                    ps_s[:, bass.ts(q, 256)],
                                    kt_sb[:, bass.ts(j, 128)],
                                    qt_sb[hp][:, bass.ts(i, 256)],
                                    start=True, stop=True)
                            se = asp.tile([128, 512], BF16, tag="sexp",
                                          name=f"se{i}_{hp}_{p0}")
                            nc.scalar.activation(
                                se[:, 0:w], ps_s[:, 0:w],
                                mybir.ActivationFunctionType.Exp,
                                bias=0.0, scale=float(SCALE))
                            for q, (j, mid) in enumerate(pair):
                                if mid is not None:
                                    nc.gpsimd.tensor_mul(
                                        se[:, bass.ts(q, 256)],
                                        se[:, bass.ts(q, 256)],
                                        mask_sb[:, bass.ts(mid, 256)])
                            for q, (j, mid) in enumerate(pair):
                                jj = p0 + q
                                for m in range(2):
                                    nc.tensor.matmul(
                                        ps_o[m][:],
                                        se[:, bass.ds(256 * q + 128 * m, 128)],
                                        vaug_sb[j][:],
                                        start=(jj == 0),
                                        stop=(jj == njobs - 1))
                        for m in range(2):
                            h = 2 * hp + m
                            recip = asmall.tile([128, 1], F32, tag="recip",
                                                name=f"rc{i}_{h}")
                            nc.vector.reciprocal(recip[:],
                                                 ps_o[m][:, D : D + 1])
                            a_n = asp.tile([128, 128], BF16, tag="anorm",
                                           name=f"an{i}_{h}")
                            nc.vector.tensor_scalar_mul(a_n[:],
                                                        ps_o[m][:, 0:D],
                                                        recip[:])
                            at_p = spp.tile([128, 128], BF16, tag="score",
                                            name=f"atp{i}_{h}")
                            nc.tensor.transpose(at_p[:], a_n[:], ident_sb[:])
                            at = asp.tile([128, 128], BF16, tag="at",
                                          bufs=10, name=f"at{i}_{h}")
                            nc.vector.tensor_copy(at[:], at_p[:])
                            # quantize to e4m3 + residual (values carry SA)
                            nc.vector.tensor_copy(at8_pair[hp][:, m], at[:])
                            nc.vector.tensor_sub(rat8_pair[hp][:, m], at[:],
                                                 at8_pair[hp][:, m])

                    if prev_at is not None:
                        oproj(i - 1, *prev_at)
                    prev_at = (at8_pair, rat8_pair)
                oproj(NT - 1, *prev_at)

    nc.compile()
    return nc


def _get_nc(cu_seqlens):
    key = np.asarray(cu_seqlens).tobytes()
    if key not in _cache:
        jobs, masks_np, cos_t, sin_s, ident = _host_prep(cu_seqlens)
        nc = _build(jobs, masks_np.shape[1] // 256)
        _cache[key] = (nc, masks_np, cos_t, sin_s, ident)
    return _cache[key]


def kernel(hidden_states, Wq, Wk, Wv, Wo, cu_seqlens):
    hidden_states = np.asarray(hidden_states)
    Wq, Wk, Wv, Wo = (np.asarray(a) for a in (Wq, Wk, Wv, Wo))
    cu_seqlens = np.asarray(cu_seqlens)
    nc, masks_np, cos_t, sin_s, ident = _get_nc(cu_seqlens)

    # hT in fp8: base + residual at scale SX
    ht = np.ascontiguousarray(hidden_states.T) * np.float32(SX)
    h8, hr8 = _q8(ht)
    # tile for contiguous DMA: [NSTRIP, NKT//4, 128, 2, 4, 512] — each
    # DMA carries 4 hidden k-tiles of x8 then the matching rx8
    def tile_ht(a):
        return np.ascontiguousarray(
            a.reshape(NKT // 4, 4, 128, NSTRIP, 512).transpose(3, 0, 2, 1, 4)
        ).reshape(NSTRIP, NKT // 4, 128, 1, 4, 512)
    ht_tiled = np.concatenate([tile_ht(h8), tile_ht(hr8)], axis=3)

    in_maps = []
    for c in range(N_CORES):
        # flat per-tensor lhsT layout: [128, HID] with col = 128*ktile + m
        def tile_wq(a):
            return np.ascontiguousarray(
                a.reshape(NKT, 128, HPC, 128).transpose(2, 1, 0, 3)
            ).reshape(HPC, 128, HID)
        def tile_wkv(a):
            return np.ascontiguousarray(
                a.reshape(NKT, 128, 128).transpose(1, 0, 2)).reshape(128, HID)
        wq_c = Wq[:, QD * c : QD * (c + 1)].astype(np.float32) * np.float32(SW)
        wq8, wqr = _q8(wq_c)
        wk8, wkr = _q8(Wk[:, D * c : D * (c + 1)].astype(np.float32)
                       * np.float32(SW))
        wv8, wvr = _q8(Wv[:, D * c : D * (c + 1)].astype(np.float32)
                       * np.float32(SW))
        # pack the 12 tensors group-interleaved:
        # wall[g, :, t, :] = flat_t[:, 512g : 512(g+1)]
        flats = ([tile_wq(wq8)[h] for h in range(HPC)]
                 + [tile_wkv(wk8), tile_wkv(wv8)]
                 + [tile_wq(wqr)[h] for h in range(HPC)]
                 + [tile_wkv(wkr), tile_wkv(wvr)])
        stackw = np.stack(flats)  # [12, 128, HID]
        wall = np.ascontiguousarray(
            stackw.reshape(12, 128, NKT // 4, 512).transpose(2, 1, 0, 3)
        ).reshape(NKT // 4, 128, 12, 2, 2, 128)
        # wo: [2, hpair, 128 d, 2 head-in-pair, HID]
        wo_c = Wo[QD * c : QD * (c + 1), :].astype(np.float32) \
            * np.float32(SWO)
        wo8, wor = _q8(wo_c)
        def tile_wo(a):
            return np.ascontiguousarray(
                a.reshape(2, 2, 128, HID).transpose(0, 2, 1, 3))
        wo_t = np.stack([tile_wo(wo8), tile_wo(wor)])
        in_maps.append({
            "ht": ht_tiled, "wall": wall, "wo": wo_t,
            "cos_t": cos_t, "sin_s": sin_s, "ident": ident,
            "masks": masks_np,
        })

    res = bass_utils.run_bass_kernel_spmd(nc, in_maps,
                                          core_ids=list(range(N_CORES)))
    out = res.results[0]["out"].astype(np.float64)
    for c in range(1, N_CORES):
        out += res.results[c]["out"].astype(np.float64)
    return out.astype(np.float32)
